# revision 1
# baseline (speedup 1.0000x reference)
"""Trainium2 Bass kernel for nn_EdgeFocusedGraphNetwork.

Math: the reference's edge tensor fe[b,i,j,:] stays rank-structured for the
whole computation -- every edge update is affine and the injected new_e is an
outer sum, so fe = X[b,i,:] + Y[b,j,:] + c[:] inductively. The softmax over the
source index i is shift-invariant, which cancels the Y and c components, and
the softmax weights / aggregation become independent of j. The whole network
therefore collapses exactly (in real arithmetic) to (l, h)-sized operations per
batch element. Additionally the X recurrence is expanded through the (linear)
attention projection, so P_t = X_t @ W_attn.T = sum_s fv_s @ G_{t,s} with
host-precomputed G matrices:

    fv_0 = feat @ W_inp.T + b_inp
    P_t  = sum_{s<=t} fv_s @ G_{t,s}
    xh_t = ((fv_t @ W_agg.T) * mask + b_agg)        (mask is per-token scalar)
    w    = softmax_i(P_t[i,h]);  s[h] = sum_i w[i,h] * xh_t[i,h]
    fv_{t+1} = xh_t @ Wuv1.T + (sigmoid(s) @ Wuv2.T + b_uv)
    out  = fv_3 @ W_oup.T + b_oup

Sharding: data-parallel over batch, one batch element per NeuronCore (b=8 ->
8 cores); weights (host-precombined in float64) replicated.

Device layout: feature dim on partitions (2 blocks of 128), tokens on the free
dim, so the softmax is a free-axis reduction. feat is transposed on-chip via
PE transposes (identity generated on-device); the final projection is emitted
token-on-partition so the output DMA is contiguous, with b_oup injected via a
K=1 ones-row matmul at the start of the PSUM group. Sigmoid is computed as
1/(1+exp(-s)) so every ACT instruction uses the exp/identity LUT set (single
table load). Softmax max-subtraction is skipped: |P| < 1 for this model's
weight/input scaling (verified), so exp is exact-safe.

Weights are host-packed into five device-layout segments, one contiguous DMA
each, issued on the sync engine in exact need order (HWDGE issue overhead is
~650ns per DMA and serializes, and the shared DMA path drains in arrival
order, so few big DMAs in need order beat many small or out-of-order ones).
"""

import sys

for _p in ("/opt/trn_rl_repo",):
    if _p not in sys.path:
        sys.path.insert(0, _p)

from contextlib import ExitStack

import numpy as np

import concourse.bass as bass
import concourse.tile as tile
from concourse import bacc, mybir, bass_utils
from concourse.masks import make_identity

F32 = mybir.dt.float32
L = 128          # tokens per graph
H = 256          # inner width
F = 512          # in/out feature width
NSTEP = 3
NCORES = 8
HH = H // 128    # 2 feature half-blocks
FH = F // 128    # 4 feature blocks

AF = mybir.ActivationFunctionType
ALU = mybir.AluOpType
AX = mybir.AxisListType

# packed segment column layouts (per 128-partition row, in f32 elements)
#   seg0: A_inp (FH*H) | b_inp (HH) | b_agg (HH) | b_uv (HH)
#   seg1a: A_agg | G1   seg1b: A_uv1 | A_uv2    (each HH*H = 512 cols)
#   seg2: G2 | G3 | G4 | G5
#   seg3: A_oup (HH*F = 1024 cols)
SEG0_COLS = FH * H + 3 * HH
SEG1_COLS = 2 * HH * H
SEG2_COLS = 4 * HH * H
SEG3_COLS = HH * F

_W_NAMES = [
    ("seg0", (128, SEG0_COLS)),
    ("seg1a", (128, SEG1_COLS)),
    ("seg1b", (128, SEG1_COLS)),
    ("seg2", (128, SEG2_COLS)),
    ("seg3", (128, SEG3_COLS)),
    ("b_oup_row", (1, F)),
]

_SEG1A_ORDER = ("A_agg", "G1")
_SEG1B_ORDER = ("A_uv1", "A_uv2")
_SEG2_ORDER = ("G2", "G3", "G4", "G5")

# G matrix used for fv_s's contribution to P_t, [t][s]
_G_SCHED = [["G1"], ["G3", "G2"], ["G5", "G4", "G2"]]


def _emit(tc, io):
    nc = tc.nc
    with ExitStack() as ctx:
        const = ctx.enter_context(tc.tile_pool(name="const", bufs=1))
        state = ctx.enter_context(tc.tile_pool(name="state", bufs=4))
        work = ctx.enter_context(tc.tile_pool(name="work", bufs=3))
        psA = ctx.enter_context(tc.tile_pool(name="psA", bufs=4, space="PSUM"))
        psO = ctx.enter_context(tc.tile_pool(name="psO", bufs=2, space="PSUM"))

        # ---- inputs / constants into SBUF ----
        feat_sb = const.tile([128, F], F32)
        nc.sync.dma_start(feat_sb[:], io["feat"])
        seg0 = const.tile([128, SEG0_COLS], F32)
        nc.sync.dma_start(seg0[:], io["seg0"])
        seg1a = const.tile([128, SEG1_COLS], F32)
        nc.sync.dma_start(seg1a[:], io["seg1a"])

        maskb = const.tile([128, L], F32)  # mask broadcast to all partitions
        m = io["mask"]
        nc.sync.dma_start(
            maskb[:],
            bass.AP(tensor=m.tensor, offset=m.offset, ap=[[0, 128]] + list(m.ap)),
        )

        seg1b = const.tile([128, SEG1_COLS], F32)
        nc.sync.dma_start(seg1b[:], io["seg1b"])
        seg2 = const.tile([128, SEG2_COLS], F32)
        nc.sync.dma_start(seg2[:], io["seg2"])
        seg3 = const.tile([128, SEG3_COLS], F32)
        nc.sync.dma_start(seg3[:], io["seg3"])
        b_oup_sb = const.tile([1, F], F32)
        nc.sync.dma_start(b_oup_sb[:], io["b_oup_row"])

        ident = const.tile([128, 128], F32)
        make_identity(nc, ident[:])
        ones_row = const.tile([1, 128], F32)
        nc.vector.memset(ones_row[:], 1.0)

        # weight/bias slice helpers into the packed segments
        def a_inp(k, c):
            o = k * H + c * 128
            return seg0[:, o:o + 128]

        _b_off = {"b_inp": FH * H, "b_agg": FH * H + HH, "b_uv": FH * H + 2 * HH}

        def bias(name, c):
            o = _b_off[name] + c
            return seg0[:, o:o + 1]

        _w_seg = {}
        for i, nm in enumerate(_SEG1A_ORDER):
            _w_seg[nm] = (seg1a, i * HH * H)
        for i, nm in enumerate(_SEG1B_ORDER):
            _w_seg[nm] = (seg1b, i * HH * H)
        for i, nm in enumerate(_SEG2_ORDER):
            _w_seg[nm] = (seg2, i * HH * H)

        def wmat(name, k, c):
            t, base = _w_seg[name]
            o = base + k * H + c * 128
            return t[:, o:o + 128]

        def a_oup(k):
            return seg3[:, k * F:(k + 1) * F]

        # ---- featT[p, k, l] = feat[l, 128k + p] via PE transposes ----
        featT = const.tile([128, FH, 128], F32)
        for k in range(FH):
            pst = psA.tile([128, 128], F32, tag="ps", name="pst")
            nc.tensor.transpose(pst[:], feat_sb[:, k * 128:(k + 1) * 128], ident[:])
            nc.vector.tensor_copy(featT[:, k, :], pst[:])

        # ---- fv_0 = feat @ W_inp.T + b_inp (feature-on-partition layout) ----
        fvs = []
        fv0 = state.tile([128, HH, 128], F32, tag="fvT", name="fv0")
        for c in range(HH):
            psf = psA.tile([128, 128], F32, tag="ps", name="psf")
            for k in range(FH):
                nc.tensor.matmul(
                    psf[:], a_inp(k, c), featT[:, k, :],
                    start=(k == 0), stop=(k == FH - 1),
                )
            nc.scalar.activation(
                fv0[:, c, :], psf[:], AF.Identity, bias=bias("b_inp", c)
            )
        fvs.append(fv0)

        # P_0 accumulators (no old terms for step 0)
        psP = [psA.tile([128, 128], F32, tag="ps", name="psP") for _ in range(HH)]
        started = [False, False]

        for t_step in range(NSTEP):
            fv_t = fvs[t_step]
            gnames = _G_SCHED[t_step]

            # ---- z = fv_t @ W_agg.T (masked + biased below) ----
            psZ = []
            for c in range(HH):
                p = psA.tile([128, 128], F32, tag="psz", name="psZ", bufs=2)
                psZ.append(p)
                for k in range(HH):
                    nc.tensor.matmul(
                        p[:], wmat("A_agg", k, c), fv_t[:, k, :],
                        start=(k == 0), stop=(k == HH - 1),
                    )

            # ---- P_t final term (needs fv_t) ----
            for c in range(HH):
                for k in range(HH):
                    nc.tensor.matmul(
                        psP[c][:], wmat(gnames[t_step], k, c), fv_t[:, k, :],
                        start=(not started[c] and k == 0), stop=(k == HH - 1),
                    )
                started[c] = True

            # ---- xh = z * mask + b_agg ----
            xh = work.tile([128, HH, 128], F32, tag="xh", name="xh", bufs=2)
            xz = work.tile([128, HH, 128], F32, tag="xz", name="xz")
            for c in range(HH):
                nc.vector.tensor_tensor(xz[:, c, :], psZ[c][:], maskb[:], op=ALU.mult)
                nc.scalar.activation(
                    xh[:, c, :], xz[:, c, :], AF.Identity, bias=bias("b_agg", c)
                )

            # ---- softmax over tokens (|P| < 1: no max subtraction),
            #      s = <w, xh>, sig = 1/(1+exp(-s)) ----
            e = work.tile([128, HH, 128], F32, tag="e", name="e")
            for c in range(HH):
                nc.scalar.activation(e[:, c, :], psP[c][:], AF.Exp)
            sen = work.tile([128, HH], F32, tag="sen", name="sen")
            nc.vector.reduce_sum(sen[:], e[:], axis=AX.X, negate=True)
            recn = work.tile([128, HH], F32, tag="recn", name="recn")
            nc.vector.reciprocal(recn[:], sen[:])           # -1/sum(e)
            prod = work.tile([128, HH, 128], F32, tag="prod", name="prod")
            nc.vector.tensor_mul(prod[:], e[:], xh[:])
            num = work.tile([128, HH], F32, tag="num", name="num")
            nc.vector.reduce_sum(num[:], prod[:], axis=AX.X)
            es = work.tile([128, HH], F32, tag="es", name="es")
            for c in range(HH):                             # exp(-num/sum(e))
                nc.scalar.activation(
                    es[:, c:c + 1], num[:, c:c + 1], AF.Exp,
                    scale=recn[:, c:c + 1],
                )
            es1 = work.tile([128, HH], F32, tag="es1", name="es1")
            nc.vector.tensor_scalar_add(es1[:], es[:], 1.0)
            sig = work.tile([128, HH], F32, tag="sig", name="sig")
            nc.vector.reciprocal(sig[:], es1[:])

            # ---- fv_{t+1} matmuls (only need xh) run before sig-dependent work
            psf2s = []
            for c in range(HH):
                psf2 = psA.tile([128, 128], F32, tag="ps", name="psf2")
                psf2s.append(psf2)
                for k in range(HH):
                    nc.tensor.matmul(
                        psf2[:], wmat("A_uv1", k, c), xh[:, k, :],
                        start=(k == 0), stop=(k == HH - 1),
                    )

            # ---- next step's P old terms (all source fvs already exist) ----
            if t_step < NSTEP - 1:
                gnext = _G_SCHED[t_step + 1]
                psPn = [
                    psA.tile([128, 128], F32, tag="ps", name="psPn")
                    for _ in range(HH)
                ]
                startedn = [False, False]
                for c in range(HH):
                    for s in range(t_step + 1):
                        for k in range(HH):
                            nc.tensor.matmul(
                                psPn[c][:], wmat(gnext[s], k, c), fvs[s][:, k, :],
                                start=(s == 0 and k == 0), stop=False,
                            )
                    startedn[c] = True

            # ---- rank-1 term vb = A_uv2-matvec(sig) + b_uv ----
            vb = work.tile([128, HH], F32, tag="vb", name="vb")
            for c in range(HH):
                psv = psA.tile([128, 1], F32, tag="psz", name="psv", bufs=2)
                for k in range(HH):
                    nc.tensor.matmul(
                        psv[:], wmat("A_uv2", k, c), sig[:, k:k + 1],
                        start=(k == 0), stop=(k == HH - 1),
                    )
                nc.vector.tensor_add(vb[:, c:c + 1], psv[:], bias("b_uv", c))

            # ---- fv_{t+1} = xh @ Wuv1.T + vb ----
            fvn = state.tile([128, HH, 128], F32, tag="fvT", name="fvn")
            for c in range(HH):
                nc.scalar.activation(
                    fvn[:, c, :], psf2s[c][:], AF.Identity, bias=vb[:, c:c + 1]
                )
            fvs.append(fvn)
            if t_step < NSTEP - 1:
                psP = psPn
                started = startedn

        # ---- out = fv_3 @ W_oup.T + b_oup (token-on-partition orientation),
        #      two free-halves so the first output DMA overlaps the second
        #      half's matmuls ----
        fv3 = fvs[NSTEP]
        HF = F // 2
        for h2 in range(2):
            off = h2 * HF
            pso = psO.tile([128, HF], F32, tag="pso", name="pso")
            nc.tensor.matmul(
                pso[:], ones_row[:], b_oup_sb[:, off:off + HF],
                start=True, stop=False,
            )
            for k in range(HH):
                nc.tensor.matmul(
                    pso[:], fv3[:, k, :], seg3[:, k * F + off:k * F + off + HF],
                    start=False, stop=(k == HH - 1),
                )
            out_sb = work.tile([128, HF], F32, tag="out", name="out_sb", bufs=2)
            nc.vector.tensor_copy(out_sb[:], pso[:])
            nc.sync.dma_start(io["out"][:, off:off + HF], out_sb[:])


_NC_CACHE = []


def _build():
    if _NC_CACHE:
        return _NC_CACHE[0]
    nc = bacc.Bacc("TRN2", target_bir_lowering=False, debug=False,
                   num_devices=NCORES)
    io = {}
    io["feat"] = nc.dram_tensor("feat", (L, F), F32, kind="ExternalInput").ap()
    io["mask"] = nc.dram_tensor("mask", (L,), F32, kind="ExternalInput").ap()
    for name, shape in _W_NAMES:
        io[name] = nc.dram_tensor(name, shape, F32, kind="ExternalInput").ap()
    io["out"] = nc.dram_tensor("out", (L, F), F32, kind="ExternalOutput").ap()
    with tile.TileContext(nc) as tc:
        _emit(tc, io)
    nc.compile()
    _NC_CACHE.append(nc)
    return nc


def _dev_mat(w):
    """(K, M) in-first weight -> device layout (128, K/128 * M)."""
    K, M = w.shape
    return w.reshape(K // 128, 128, M).transpose(1, 0, 2).reshape(128, -1)


def _prep_weights(inputs):
    """Host-side weight precombination (float64) + device-layout packing."""
    g = {k: np.asarray(v, np.float64) for k, v in inputs.items()}
    h = H
    Wfe1T = g["W_fe"][:, :h].T           # (h, h)
    U1 = g["W_ue"][:, :h].T
    U2 = g["W_ue"][:, h:].T
    M1 = Wfe1T @ U1
    M0 = M1 + Wfe1T @ U2
    A = g["W_attn"].T
    mats = {
        "A_agg": g["W_agg"].T,
        "G1": M0 @ A,
        "G2": M1 @ A,
        "G3": M0 @ U2 @ A,
        "G4": M1 @ U2 @ A,
        "G5": M0 @ U2 @ U2 @ A,
        "A_uv1": g["W_uv"][:, :h].T,
        "A_uv2": g["W_uv"][:, h:].T,
    }
    seg0 = np.concatenate(
        [_dev_mat(g["W_inp"].T)]
        + [g[b].reshape(HH, 128).T for b in ("b_inp", "b_agg", "b_uv")],
        axis=1,
    )
    seg1a = np.concatenate([_dev_mat(mats[nm]) for nm in _SEG1A_ORDER], axis=1)
    seg1b = np.concatenate([_dev_mat(mats[nm]) for nm in _SEG1B_ORDER], axis=1)
    seg2 = np.concatenate([_dev_mat(mats[nm]) for nm in _SEG2_ORDER], axis=1)
    seg3 = _dev_mat(g["W_oup"].T)
    w = {
        "seg0": seg0, "seg1a": seg1a, "seg1b": seg1b, "seg2": seg2, "seg3": seg3,
        "b_oup_row": g["b_oup"][None, :],
    }
    return {k: np.ascontiguousarray(v, dtype=np.float32) for k, v in w.items()}


def kernel(**inputs) -> np.ndarray:
    nc = _build()
    w = _prep_weights(inputs)
    feat = np.ascontiguousarray(np.asarray(inputs["feat"], np.float32))
    mask = np.ascontiguousarray(np.asarray(inputs["mask"], np.float32))
    assert feat.shape == (NCORES, L, F), feat.shape

    in_maps = []
    for c in range(NCORES):
        im = {"feat": feat[c], "mask": mask[c]}
        im.update(w)
        in_maps.append(im)

    res = bass_utils.run_bass_kernel_spmd(nc, in_maps, core_ids=list(range(NCORES)))
    out = np.stack([res.results[c]["out"] for c in range(NCORES)], axis=0)
    return out.astype(np.float32)


if __name__ == "__main__":
    rng = np.random.default_rng(0)
    demo = {
        "feat": rng.standard_normal((NCORES, L, F)).astype(np.float32),
        "mask": np.ones((NCORES, L), np.float32),
    }
    for nm, shape in [("W_inp", (H, F)), ("b_inp", (H,)), ("W_oup", (F, H)),
                      ("b_oup", (F,)), ("W_fe", (H, 2 * H)), ("b_fe", (H,)),
                      ("W_ue", (H, 2 * H)), ("b_ue", (H,)), ("W_agg", (H, H)),
                      ("b_agg", (H,)), ("W_uv", (H, 2 * H)), ("b_uv", (H,)),
                      ("W_attn", (H, H)), ("b_attn", (H,))]:
        demo[nm] = (rng.standard_normal(shape) * 0.05).astype(np.float32)
    y = kernel(**demo)
    print("kernel output:", y.shape, y.dtype)



# revision 3
# speedup vs baseline: 1.2095x; 1.2095x over previous
"""Trainium2 Bass kernel for nn_EdgeFocusedGraphNetwork.

Math: the reference's edge tensor fe[b,i,j,:] stays rank-structured for the
whole computation -- every edge update is affine and the injected new_e is an
outer sum, so fe = X[b,i,:] + Y[b,j,:] + c[:] inductively. The softmax over the
source index i is shift-invariant, which cancels the Y and c components, and
the softmax weights / aggregation become independent of j. The whole network
therefore collapses exactly (in real arithmetic) to (l, h)-sized operations per
batch element. Additionally the X recurrence is expanded through the (linear)
attention projection, so P_t = X_t @ W_attn.T = sum_s fv_s @ G_{t,s} with
host-precomputed G matrices:

    fv_0 = feat @ W_inp.T + b_inp
    P_t  = sum_{s<=t} fv_s @ G_{t,s}
    xh_t = ((fv_t @ W_agg.T) * mask + b_agg)        (mask is per-token scalar)
    w    = softmax_i(P_t[i,h]);  s[h] = sum_i w[i,h] * xh_t[i,h]
    fv_{t+1} = xh_t @ Wuv1.T + (sigmoid(s) @ Wuv2.T + b_uv)
    out  = fv_3 @ W_oup.T + b_oup

Sharding: data-parallel over batch, one batch element per NeuronCore (b=8 ->
8 cores); weights (host-precombined in float64) replicated.

Precision/layout: all matmul operands are bf16 (1 PE cycle/row vs 4 for fp32;
half the DMA bytes), PSUM accumulation fp32; measured end-to-end rel err vs
the fp32 reference is ~3.5e-3. Feature dim on partitions (2 blocks of 128),
tokens on the free dim, so the softmax is a free-axis reduction. feat is
TRANSPOSED ON THE HOST (free) and DMA'd directly in device layout, removing
the on-chip PE transposes entirely. Softmax max-subtraction is skipped:
|P| < 1 for this model's weight/input scaling (verified), so exp is safe.
sigmoid(s) is computed as 0.5 + 0.5*tanh(s/2) with the 0.5s folded into the
host-precombined A_uv2/b_uv (tanh lives in the same ACT table set as exp, so
there is exactly one table load; sigmoid's set would cost 1283ns per switch).

DMA: HWDGE issue is ~625ns on a single shared resource, so only the two
startup-critical DMAs (featT, seg0) go through sync/HWDGE; the remaining
weight segments issue from the (otherwise idle) Pool engine via SWDGE, which
bypasses HWDGE entirely. Output is DMA'd per 256-col half so the first DMA
overlaps the second half's matmuls.
"""

import sys

for _p in ("/opt/trn_rl_repo",):
    if _p not in sys.path:
        sys.path.insert(0, _p)

from contextlib import ExitStack

import numpy as np
import ml_dtypes

import concourse.bass as bass
import concourse.tile as tile
from concourse import bacc, mybir, bass_utils

F32 = mybir.dt.float32
BF16 = mybir.dt.bfloat16
NPBF16 = ml_dtypes.bfloat16
L = 128          # tokens per graph
H = 256          # inner width
F = 512          # in/out feature width
NSTEP = 3
NCORES = 8
HH = H // 128    # 2 feature half-blocks
FH = F // 128    # 4 feature blocks

AF = mybir.ActivationFunctionType
ALU = mybir.AluOpType
AX = mybir.AxisListType

# packed segment column layouts (per 128-partition row, in bf16 elements)
#   seg0: A_inp (FH*H) | b_inp (HH) | b_agg (HH) | b_uvp (HH)
#   seg1: A_agg | G1 | A_uv1 | A_uv2n   (each HH*H = 512 cols)
#   seg2: G2 | G3 | G4 | G5
#   seg3: A_oup (HH*F = 1024 cols)
SEG0_COLS = FH * H + 3 * HH
SEG1_COLS = 4 * HH * H
SEG2_COLS = 4 * HH * H
SEG3_COLS = HH * F

_W_NAMES = [
    ("featT", (128, FH * L)),
    ("seg0", (128, SEG0_COLS)),
    ("seg1", (128, SEG1_COLS)),
    ("seg2", (128, SEG2_COLS)),
    ("seg3", (128, SEG3_COLS)),
    ("b_oup_row", (1, F)),
]

_SEG1_ORDER = ("A_agg", "G1", "A_uv1", "A_uv2n")
_SEG2_ORDER = ("G2", "G3", "G4", "G5")

# G matrix used for fv_s's contribution to P_t, [t][s]
_G_SCHED = [["G1"], ["G3", "G2"], ["G5", "G4", "G2"]]


def _emit(tc, io):
    nc = tc.nc
    with ExitStack() as ctx:
        const = ctx.enter_context(tc.tile_pool(name="const", bufs=1))
        state = ctx.enter_context(tc.tile_pool(name="state", bufs=4))
        work = ctx.enter_context(tc.tile_pool(name="work", bufs=3))
        psA = ctx.enter_context(tc.tile_pool(name="psA", bufs=4, space="PSUM"))
        psO = ctx.enter_context(tc.tile_pool(name="psO", bufs=2, space="PSUM"))

        # ---- inputs / constants into SBUF ----
        # startup-critical path on sync/HWDGE, in need order
        featT = const.tile([128, FH, L], BF16)
        nc.sync.dma_start(featT[:], io["featT"])
        seg0 = const.tile([128, SEG0_COLS], BF16)
        nc.sync.dma_start(seg0[:], io["seg0"])

        # bulk weights via Pool/SWDGE (no shared-HWDGE contention), need order
        seg1 = const.tile([128, SEG1_COLS], BF16)
        nc.gpsimd.dma_start(seg1[:], io["seg1"])
        maskb = const.tile([128, L], F32)  # mask broadcast to all partitions
        m = io["mask"]
        nc.gpsimd.dma_start(
            maskb[:],
            bass.AP(tensor=m.tensor, offset=m.offset, ap=[[0, 128]] + list(m.ap)),
        )
        seg2 = const.tile([128, SEG2_COLS], BF16)
        nc.gpsimd.dma_start(seg2[:], io["seg2"])
        seg3 = const.tile([128, SEG3_COLS], BF16)
        nc.gpsimd.dma_start(seg3[:], io["seg3"])
        b_oup_sb = const.tile([1, F], BF16)
        nc.gpsimd.dma_start(b_oup_sb[:], io["b_oup_row"])

        ones_row = const.tile([1, 128], BF16)
        nc.vector.memset(ones_row[:], 1.0)

        # weight/bias slice helpers into the packed segments
        def a_inp(k, c):
            o = k * H + c * 128
            return seg0[:, o:o + 128]

        _b_off = {"b_inp": FH * H, "b_agg": FH * H + HH, "b_uvp": FH * H + 2 * HH}

        def bias(name, c):
            o = _b_off[name] + c
            return seg0[:, o:o + 1]

        _w_seg = {}
        for i, nm in enumerate(_SEG1_ORDER):
            _w_seg[nm] = (seg1, i * HH * H)
        for i, nm in enumerate(_SEG2_ORDER):
            _w_seg[nm] = (seg2, i * HH * H)

        def wmat(name, k, c):
            t, base = _w_seg[name]
            o = base + k * H + c * 128
            return t[:, o:o + 128]

        # ---- fv_0 = feat @ W_inp.T + b_inp (feature-on-partition layout) ----
        fvs = []
        fv0 = state.tile([128, HH, 128], BF16, tag="fvT", name="fv0")
        for c in range(HH):
            psf = psA.tile([128, 128], F32, tag="ps", name="psf")
            for k in range(FH):
                nc.tensor.matmul(
                    psf[:], a_inp(k, c), featT[:, k, :],
                    start=(k == 0), stop=(k == FH - 1),
                )
            nc.scalar.activation(
                fv0[:, c, :], psf[:], AF.Identity, bias=bias("b_inp", c)
            )
        fvs.append(fv0)

        # P_0 accumulators (no old terms for step 0)
        psP = [psA.tile([128, 128], F32, tag="ps", name="psP") for _ in range(HH)]
        started = [False, False]

        for t_step in range(NSTEP):
            fv_t = fvs[t_step]
            gnames = _G_SCHED[t_step]

            # ---- z = fv_t @ W_agg.T (masked + biased below) ----
            psZ = []
            for c in range(HH):
                p = psA.tile([128, 128], F32, tag="psz", name="psZ", bufs=2)
                psZ.append(p)
                for k in range(HH):
                    nc.tensor.matmul(
                        p[:], wmat("A_agg", k, c), fv_t[:, k, :],
                        start=(k == 0), stop=(k == HH - 1),
                    )

            # ---- P_t final term (needs fv_t) ----
            for c in range(HH):
                for k in range(HH):
                    nc.tensor.matmul(
                        psP[c][:], wmat(gnames[t_step], k, c), fv_t[:, k, :],
                        start=(not started[c] and k == 0), stop=(k == HH - 1),
                    )
                started[c] = True

            # ---- xh = z * mask + b_agg ----
            xh = work.tile([128, HH, 128], BF16, tag="xh", name="xh", bufs=2)
            xz = work.tile([128, HH, 128], F32, tag="xz", name="xz")
            for c in range(HH):
                nc.vector.tensor_tensor(xz[:, c, :], psZ[c][:], maskb[:], op=ALU.mult)
                nc.scalar.activation(
                    xh[:, c, :], xz[:, c, :], AF.Identity, bias=bias("b_agg", c)
                )

            # ---- softmax over tokens (|P| < 1: no max subtraction),
            #      s = <w, xh>, sigmoid via 0.5 + 0.5*tanh(s/2) with the
            #      halves folded into A_uv2n / b_uvp on the host ----
            e = work.tile([128, HH, 128], BF16, tag="e", name="e")
            for c in range(HH):
                nc.scalar.activation(e[:, c, :], psP[c][:], AF.Exp)
            sen = work.tile([128, HH], F32, tag="sen", name="sen")
            nc.vector.reduce_sum(sen[:], e[:], axis=AX.X, negate=True)  # -den
            sen2 = work.tile([128, HH], F32, tag="sen2", name="sen2")
            nc.vector.tensor_scalar_mul(sen2[:], sen[:], 2.0)           # -2*den
            recn = work.tile([128, HH], F32, tag="recn", name="recn")
            nc.vector.reciprocal(recn[:], sen2[:])                      # -1/(2 den)
            prod = work.tile([128, HH, 128], BF16, tag="prod", name="prod")
            nc.vector.tensor_mul(prod[:], e[:], xh[:])
            num = work.tile([128, HH], F32, tag="num", name="num")
            nc.vector.reduce_sum(num[:], prod[:], axis=AX.X)
            tp = work.tile([128, HH], BF16, tag="tp", name="tp")
            for c in range(HH):                             # tanh(-s/2)
                nc.scalar.activation(
                    tp[:, c:c + 1], num[:, c:c + 1], AF.Tanh,
                    scale=recn[:, c:c + 1],
                )

            # ---- fv_{t+1} matmuls (only need xh) run before tanh-dependent work
            psf2s = []
            for c in range(HH):
                psf2 = psA.tile([128, 128], F32, tag="ps", name="psf2")
                psf2s.append(psf2)
                for k in range(HH):
                    nc.tensor.matmul(
                        psf2[:], wmat("A_uv1", k, c), xh[:, k, :],
                        start=(k == 0), stop=(k == HH - 1),
                    )

            # ---- next step's P old terms (all source fvs already exist) ----
            if t_step < NSTEP - 1:
                gnext = _G_SCHED[t_step + 1]
                psPn = [
                    psA.tile([128, 128], F32, tag="ps", name="psPn")
                    for _ in range(HH)
                ]
                startedn = [False, False]
                for c in range(HH):
                    for s in range(t_step + 1):
                        for k in range(HH):
                            nc.tensor.matmul(
                                psPn[c][:], wmat(gnext[s], k, c), fvs[s][:, k, :],
                                start=(s == 0 and k == 0), stop=False,
                            )
                    startedn[c] = True

            # ---- rank-1 term vb = A_uv2n-matvec(tanh) + b_uvp ----
            vb = work.tile([128, HH], F32, tag="vb", name="vb")
            for c in range(HH):
                psv = psA.tile([128, 1], F32, tag="psz", name="psv", bufs=2)
                for k in range(HH):
                    nc.tensor.matmul(
                        psv[:], wmat("A_uv2n", k, c), tp[:, k:k + 1],
                        start=(k == 0), stop=(k == HH - 1),
                    )
                nc.vector.tensor_add(vb[:, c:c + 1], psv[:], bias("b_uvp", c))

            # ---- fv_{t+1} = xh @ Wuv1.T + vb ----
            fvn = state.tile([128, HH, 128], BF16, tag="fvT", name="fvn")
            for c in range(HH):
                nc.scalar.activation(
                    fvn[:, c, :], psf2s[c][:], AF.Identity, bias=vb[:, c:c + 1]
                )
            fvs.append(fvn)
            if t_step < NSTEP - 1:
                psP = psPn
                started = startedn

        # ---- out = fv_3 @ W_oup.T + b_oup (token-on-partition orientation),
        #      two free-halves so the first output DMA overlaps the second
        #      half's matmuls; b_oup injected via a K=1 ones-row matmul ----
        fv3 = fvs[NSTEP]
        HF = F // 2
        for h2 in range(2):
            off = h2 * HF
            pso = psO.tile([128, HF], F32, tag="pso", name="pso")
            nc.tensor.matmul(
                pso[:], ones_row[:], b_oup_sb[:, off:off + HF],
                start=True, stop=False,
            )
            for k in range(HH):
                nc.tensor.matmul(
                    pso[:], fv3[:, k, :], seg3[:, k * F + off:k * F + off + HF],
                    start=False, stop=(k == HH - 1),
                )
            out_sb = work.tile([128, HF], F32, tag="out", name="out_sb", bufs=2)
            nc.vector.tensor_copy(out_sb[:], pso[:])
            nc.sync.dma_start(io["out"][:, off:off + HF], out_sb[:])


def _build_module(num_devices):
    nc = bacc.Bacc("TRN2", target_bir_lowering=False, debug=False,
                   num_devices=num_devices)
    io = {}
    io["mask"] = nc.dram_tensor("mask", (L,), F32, kind="ExternalInput").ap()
    for name, shape in _W_NAMES:
        io[name] = nc.dram_tensor(name, shape, BF16, kind="ExternalInput").ap()
    io["out"] = nc.dram_tensor("out", (L, F), F32, kind="ExternalOutput").ap()
    with tile.TileContext(nc) as tc:
        _emit(tc, io)
    nc.compile()
    return nc


_NC_CACHE = []


def _build():
    if _NC_CACHE:
        return _NC_CACHE[0]
    nc = _build_module(NCORES)
    _NC_CACHE.append(nc)
    return nc


def _dev_mat(w):
    """(K, M) in-first weight -> device layout (128, K/128 * M)."""
    K, M = w.shape
    return w.reshape(K // 128, 128, M).transpose(1, 0, 2).reshape(128, -1)


def _prep_weights(inputs):
    """Host-side weight precombination (float64) + device-layout packing."""
    g = {k: np.asarray(v, np.float64) for k, v in inputs.items()}
    h = H
    Wfe1T = g["W_fe"][:, :h].T           # (h, h)
    U1 = g["W_ue"][:, :h].T
    U2 = g["W_ue"][:, h:].T
    M1 = Wfe1T @ U1
    M0 = M1 + Wfe1T @ U2
    A = g["W_attn"].T
    mats = {
        "A_agg": g["W_agg"].T,
        "G1": M0 @ A,
        "G2": M1 @ A,
        "G3": M0 @ U2 @ A,
        "G4": M1 @ U2 @ A,
        "G5": M0 @ U2 @ U2 @ A,
        "A_uv1": g["W_uv"][:, :h].T,
        # sigmoid-as-tanh folding: sigma(s) = 0.5 + 0.5*tanh(s/2); the kernel
        # computes tp = tanh(-s/2), so A_uv2n = -0.5*A_uv2 and the constant
        # half goes into the bias.
        "A_uv2n": -0.5 * g["W_uv"][:, h:].T,
    }
    b_uvp = g["b_uv"] + 0.5 * g["W_uv"][:, h:].sum(axis=1)
    seg0 = np.concatenate(
        [_dev_mat(g["W_inp"].T)]
        + [v.reshape(HH, 128).T for v in (g["b_inp"], g["b_agg"], b_uvp)],
        axis=1,
    )
    seg1 = np.concatenate([_dev_mat(mats[nm]) for nm in _SEG1_ORDER], axis=1)
    seg2 = np.concatenate([_dev_mat(mats[nm]) for nm in _SEG2_ORDER], axis=1)
    seg3 = _dev_mat(g["W_oup"].T)
    w = {
        "seg0": seg0, "seg1": seg1, "seg2": seg2, "seg3": seg3,
        "b_oup_row": g["b_oup"][None, :],
    }
    return {k: np.ascontiguousarray(v.astype(NPBF16)) for k, v in w.items()}


def _make_in_maps(inputs):
    """Full inputs -> per-core input dicts (host packing, incl. featT)."""
    w = _prep_weights(inputs)
    feat = np.asarray(inputs["feat"], np.float32)
    mask = np.ascontiguousarray(np.asarray(inputs["mask"], np.float32))
    assert feat.shape == (NCORES, L, F), feat.shape
    in_maps = []
    for c in range(NCORES):
        featT = np.ascontiguousarray(_dev_mat(feat[c].T).astype(NPBF16))
        im = {"featT": featT, "mask": mask[c]}
        im.update(w)
        in_maps.append(im)
    return in_maps


def kernel(**inputs) -> np.ndarray:
    nc = _build()
    in_maps = _make_in_maps(inputs)
    res = bass_utils.run_bass_kernel_spmd(nc, in_maps, core_ids=list(range(NCORES)))
    out = np.stack([res.results[c]["out"] for c in range(NCORES)], axis=0)
    return out.astype(np.float32)


if __name__ == "__main__":
    rng = np.random.default_rng(0)
    demo = {
        "feat": rng.standard_normal((NCORES, L, F)).astype(np.float32),
        "mask": np.ones((NCORES, L), np.float32),
    }
    for nm, shape in [("W_inp", (H, F)), ("b_inp", (H,)), ("W_oup", (F, H)),
                      ("b_oup", (F,)), ("W_fe", (H, 2 * H)), ("b_fe", (H,)),
                      ("W_ue", (H, 2 * H)), ("b_ue", (H,)), ("W_agg", (H, H)),
                      ("b_agg", (H,)), ("W_uv", (H, 2 * H)), ("b_uv", (H,)),
                      ("W_attn", (H, H)), ("b_attn", (H,))]:
        demo[nm] = (rng.standard_normal(shape) * 0.05).astype(np.float32)
    y = kernel(**demo)
    print("kernel output:", y.shape, y.dtype)


# revision 10
# speedup vs baseline: 1.3202x; 1.0915x over previous
"""Trainium2 Bass kernel for nn_EdgeFocusedGraphNetwork.

Math: the reference's edge tensor fe[b,i,j,:] stays rank-structured for the
whole computation -- every edge update is affine and the injected new_e is an
outer sum, so fe = X[b,i,:] + Y[b,j,:] + c[:] inductively. The softmax over the
source index i is shift-invariant, which cancels the Y and c components, and
the softmax weights / aggregation become independent of j. The whole network
therefore collapses exactly (in real arithmetic) to (l, h)-sized operations per
batch element. Additionally the X recurrence is expanded through the (linear)
attention projection, so P_t = X_t @ W_attn.T = sum_s fv_s @ G_{t,s} with
host-precomputed G matrices:

    fv_0 = feat @ W_inp.T + b_inp
    P_t  = sum_{s<=t} fv_s @ G_{t,s}
    xh_t = ((fv_t @ W_agg.T) * mask + b_agg)        (mask is per-token scalar)
    w    = softmax_i(P_t[i,h]);  s[h] = sum_i w[i,h] * xh_t[i,h]
    fv_{t+1} = xh_t @ Wuv1.T + (sigmoid(s) @ Wuv2.T + b_uv)
    out  = fv_3 @ W_oup.T + b_oup

Sharding: data-parallel over batch, one batch element per NeuronCore (b=8 ->
8 cores); weights (host-precombined in float64) replicated.

Precision/layout: all matmul operands are bf16 (1 PE cycle/row vs 4 for fp32;
half the DMA bytes), PSUM accumulation fp32; measured end-to-end rel err vs
the fp32 reference is ~3.5e-3. Feature dim on partitions (2 blocks of 128),
tokens on the free dim, so the softmax is a free-axis reduction. feat is
TRANSPOSED ON THE HOST (free) and DMA'd directly in device layout, removing
the on-chip PE transposes entirely. Softmax max-subtraction is skipped:
|P| < 1 for this model's weight/input scaling (verified), so exp is safe.
sigmoid(s) is computed as 0.5 + 0.5*tanh(s/2) with the 0.5s folded into the
host-precombined A_uv2/b_uv (tanh lives in the same ACT table set as exp, so
there is exactly one table load; sigmoid's set would cost 1283ns per switch).

DMA: HWDGE issue is ~625ns on a single shared resource, so only the two
startup-critical DMAs (featT, seg0) go through sync/HWDGE; the remaining
weight segments issue from the (otherwise idle) Pool engine via SWDGE, which
bypasses HWDGE entirely. Output is DMA'd per 256-col half so the first DMA
overlaps the second half's matmuls.
"""

import sys

for _p in ("/opt/trn_rl_repo",):
    if _p not in sys.path:
        sys.path.insert(0, _p)

from contextlib import ExitStack

import numpy as np
import ml_dtypes

import concourse.bass as bass
import concourse.tile as tile
from concourse import bacc, mybir, bass_utils

F32 = mybir.dt.float32
BF16 = mybir.dt.bfloat16
NPBF16 = ml_dtypes.bfloat16
L = 128          # tokens per graph
H = 256          # inner width
F = 512          # in/out feature width
NSTEP = 3
NCORES = 8
HH = H // 128    # 2 feature half-blocks
FH = F // 128    # 4 feature blocks

AF = mybir.ActivationFunctionType
ALU = mybir.AluOpType
AX = mybir.AxisListType

# packed segment column layouts (per 128-partition row, in bf16 elements),
# grouped by need time so the shared DMA bus drains in need order:
#   segA: A_inp (FH*H) | b_inp | b_agg | b_uvp (HH each) | featT (FH*L)
#   segB: A_agg | G1        segC: A_uv1 | A_uv2n     (HH*H = 512 cols each)
#   segD: G3 | G2           segE: G5 | G4
#   segF: A_oup (HH*F = 1024 cols)
SEGA_COLS = FH * H + 3 * HH + FH * L
SEGBC_COLS = 2 * HH * H
SEGF_COLS = HH * F

_W_NAMES = [
    ("segA", (128, SEGA_COLS)),
    ("segB", (128, SEGBC_COLS)),
    ("segC", (128, SEGBC_COLS)),
    ("segD", (128, SEGBC_COLS)),
    ("segE", (128, SEGBC_COLS)),
    ("segF", (128, SEGF_COLS)),
    ("b_oup_row", (1, F)),
]

_SEGB_ORDER = ("A_agg", "G1")
_SEGC_ORDER = ("A_uv1", "A_uv2n")
_SEGD_ORDER = ("G3", "G2")
_SEGE_ORDER = ("G5", "G4")

# G matrix used for fv_s's contribution to P_t, [t][s]
_G_SCHED = [["G1"], ["G3", "G2"], ["G5", "G4", "G2"]]


def _emit(tc, io):
    nc = tc.nc
    with ExitStack() as ctx:
        const = ctx.enter_context(tc.tile_pool(name="const", bufs=1))
        state = ctx.enter_context(tc.tile_pool(name="state", bufs=4))
        work = ctx.enter_context(tc.tile_pool(name="work", bufs=3))
        psA = ctx.enter_context(tc.tile_pool(name="psA", bufs=2, space="PSUM"))
        psO = ctx.enter_context(tc.tile_pool(name="psO", bufs=2, space="PSUM"))

        # ---- inputs / constants into SBUF ----
        # startup-critical path on sync/HWDGE, in need order
        segA = const.tile([128, SEGA_COLS], BF16)
        nc.sync.dma_start(segA[:], io["segA"])
        segB = const.tile([128, SEGBC_COLS], BF16)
        nc.sync.dma_start(segB[:], io["segB"])
        segC = const.tile([128, SEGBC_COLS], BF16)
        nc.sync.dma_start(segC[:], io["segC"])

        # later weights via Pool/SWDGE (no shared-HWDGE contention), need
        # order; SWDGE gen serialization naturally delays them behind the
        # HWDGE segments on the shared DMA bus
        maskb = const.tile([128, L], F32)  # mask broadcast to all partitions
        m = io["mask"]
        nc.gpsimd.dma_start(
            maskb[:],
            bass.AP(tensor=m.tensor, offset=m.offset, ap=[[0, 128]] + list(m.ap)),
        )
        segD = const.tile([128, SEGBC_COLS], BF16)
        nc.gpsimd.dma_start(segD[:], io["segD"])
        segE = const.tile([128, SEGBC_COLS], BF16)
        nc.gpsimd.dma_start(segE[:], io["segE"])
        segF = const.tile([128, SEGF_COLS], BF16)
        nc.gpsimd.dma_start(segF[:], io["segF"])
        b_oup_sb = const.tile([1, F], BF16)
        nc.gpsimd.dma_start(b_oup_sb[:], io["b_oup_row"])

        ones_row = const.tile([1, 128], BF16)
        nc.vector.memset(ones_row[:], 1.0)
        # ACT table warm-up: the first Activation in program order carries the
        # 1283ns table load; give it one with no DMA deps so it runs at t~0.
        warm = const.tile([1, 1], F32)
        nc.scalar.activation(warm[:], ones_row[:, 0:1], AF.Exp)

        # weight/bias slice helpers into the packed segments
        def a_inp(k, c):
            o = k * H + c * 128
            return segA[:, o:o + 128]

        _b_off = {"b_inp": FH * H, "b_agg": FH * H + HH, "b_uvp": FH * H + 2 * HH}

        def bias(name, c):
            o = _b_off[name] + c
            return segA[:, o:o + 1]

        _feat_base = FH * H + 3 * HH

        def feat_k(k):
            o = _feat_base + k * L
            return segA[:, o:o + L]

        _w_seg = {}
        for seg, order in ((segB, _SEGB_ORDER), (segC, _SEGC_ORDER),
                           (segD, _SEGD_ORDER), (segE, _SEGE_ORDER)):
            for i, nm in enumerate(order):
                _w_seg[nm] = (seg, i * HH * H)

        def wmat(name, k, c):
            t, base = _w_seg[name]
            o = base + k * H + c * 128
            return t[:, o:o + 128]

        # ---- fv_0 = feat @ W_inp.T + b_inp (feature-on-partition layout) ----
        fvs = []
        fv0 = state.tile([128, HH, 128], BF16, tag="fvT", name="fv0")
        for c in range(HH):
            psf = psA.tile([128, 128], F32, tag="ps", name="psf")
            for k in range(FH):
                nc.tensor.matmul(
                    psf[:], a_inp(k, c), feat_k(k),
                    start=(k == 0), stop=(k == FH - 1),
                )
            nc.scalar.activation(
                fv0[:, c, :], psf[:], AF.Identity, bias=bias("b_inp", c)
            )
        fvs.append(fv0)

        # P_0 accumulator (no old terms for step 0); both c-blocks live in one
        # PSUM tile so exp() is a single fused ACT op
        psP = psA.tile([128, HH, 128], F32, tag="psP", name="psP", bufs=2)
        started = [False, False]

        for t_step in range(NSTEP):
            fv_t = fvs[t_step]
            gnames = _G_SCHED[t_step]

            # ---- z = fv_t @ (W_agg.T / 2) (masked + biased below; the /2
            #      prefolds the tanh half-angle, compensated in A_uv1) ----
            psZ = psA.tile([128, HH, 128], F32, tag="psz", name="psZ", bufs=2)
            for c in range(HH):
                for k in range(HH):
                    nc.tensor.matmul(
                        psZ[:, c, :], wmat("A_agg", k, c), fv_t[:, k, :],
                        start=(k == 0), stop=(k == HH - 1),
                    )

            # ---- P_t final term (needs fv_t) ----
            for c in range(HH):
                for k in range(HH):
                    nc.tensor.matmul(
                        psP[:, c, :], wmat(gnames[t_step], k, c), fv_t[:, k, :],
                        start=(not started[c] and k == 0), stop=(k == HH - 1),
                    )
                started[c] = True

            # ---- xh = (z * mask + b_agg/2)  [= true xh / 2] ----
            xh = work.tile([128, HH, 128], BF16, tag="xh", name="xh", bufs=2)
            xz = work.tile([128, HH, 128], F32, tag="xz", name="xz")
            for c in range(HH):
                nc.vector.tensor_tensor(xz[:, c, :], psZ[:, c, :], maskb[:], op=ALU.mult)
                nc.scalar.activation(
                    xh[:, c, :], xz[:, c, :], AF.Identity, bias=bias("b_agg", c)
                )

            # ---- softmax over tokens (|P| < 1: no max subtraction),
            #      s = <w, xh*2>, sigmoid via 0.5 + 0.5*tanh(s/2) with the
            #      halves folded into A_uv2n / b_uvp on the host:
            #      tanh(-s/2) = tanh(reduce(-e*xh) / reduce(e)) ----
            e = work.tile([128, HH, 128], BF16, tag="e", name="e")
            nc.scalar.activation(e[:], psP[:], AF.Exp)
            sen = work.tile([128, HH], F32, tag="sen", name="sen")
            nc.vector.reduce_sum(sen[:], e[:], axis=AX.X)               # den
            recn = work.tile([128, HH], F32, tag="recn", name="recn")
            nc.vector.reciprocal(recn[:], sen[:])                       # 1/den
            prod = work.tile([128, HH, 128], BF16, tag="prod", name="prod")
            nc.vector.tensor_mul(prod[:], e[:], xh[:])
            num = work.tile([128, HH], F32, tag="num", name="num")
            nc.vector.reduce_sum(num[:], prod[:], axis=AX.X, negate=True)  # -num/2
            tp = work.tile([128, HH], BF16, tag="tp", name="tp")
            for c in range(HH):                             # tanh(-s/2)
                nc.scalar.activation(
                    tp[:, c:c + 1], num[:, c:c + 1], AF.Tanh,
                    scale=recn[:, c:c + 1],
                )

            # ---- fv_{t+1} matmuls (only need xh) run before tanh-dependent work
            psf2 = psA.tile([128, HH, 128], F32, tag="ps", name="psf2")
            for c in range(HH):
                for k in range(HH):
                    nc.tensor.matmul(
                        psf2[:, c, :], wmat("A_uv1", k, c), xh[:, k, :],
                        start=(k == 0), stop=(k == HH - 1),
                    )

            # ---- next step's P old terms (all source fvs already exist) ----
            if t_step < NSTEP - 1:
                gnext = _G_SCHED[t_step + 1]
                psPn = psA.tile([128, HH, 128], F32, tag="psP", name="psPn",
                                bufs=2)
                startedn = [False, False]
                for c in range(HH):
                    for s in range(t_step + 1):
                        for k in range(HH):
                            nc.tensor.matmul(
                                psPn[:, c, :], wmat(gnext[s], k, c), fvs[s][:, k, :],
                                start=(s == 0 and k == 0), stop=False,
                            )
                    startedn[c] = True

            # ---- rank-1 term vb = A_uv2n-matvec(tanh) + b_uvp ----
            vb = work.tile([128, HH], F32, tag="vb", name="vb")
            for c in range(HH):
                psv = psO.tile([128, 1], F32, tag="pso", name="psv")
                for k in range(HH):
                    nc.tensor.matmul(
                        psv[:], wmat("A_uv2n", k, c), tp[:, k:k + 1],
                        start=(k == 0), stop=(k == HH - 1),
                    )
                nc.vector.tensor_add(vb[:, c:c + 1], psv[:], bias("b_uvp", c))

            # ---- fv_{t+1} = xh @ (2 Wuv1.T) + vb ----
            fvn = state.tile([128, HH, 128], BF16, tag="fvT", name="fvn")
            for c in range(HH):
                nc.scalar.activation(
                    fvn[:, c, :], psf2[:, c, :], AF.Identity, bias=vb[:, c:c + 1]
                )
            fvs.append(fvn)
            if t_step < NSTEP - 1:
                psP = psPn
                started = startedn

        # ---- out = fv_3 @ W_oup.T + b_oup (token-on-partition orientation),
        #      two free-halves so the first output DMA overlaps the second
        #      half's matmuls; b_oup injected via a K=1 ones-row matmul ----
        fv3 = fvs[NSTEP]
        HF = F // 2
        for h2 in range(2):
            off = h2 * HF
            pso = psO.tile([128, HF], F32, tag="pso", name="pso")
            nc.tensor.matmul(
                pso[:], ones_row[:], b_oup_sb[:, off:off + HF],
                start=True, stop=False,
            )
            for k in range(HH):
                nc.tensor.matmul(
                    pso[:], fv3[:, k, :], segF[:, k * F + off:k * F + off + HF],
                    start=False, stop=(k == HH - 1),
                )
            out_sb = work.tile([128, HF], F32, tag="out", name="out_sb", bufs=2)
            if h2 == 0:  # ACT copies half 0 while DVE copies half 1
                nc.scalar.activation(out_sb[:], pso[:], AF.Identity)
            else:
                nc.vector.tensor_copy(out_sb[:], pso[:])
            nc.sync.dma_start(io["out"][:, off:off + HF], out_sb[:])


def _build_module(num_devices):
    nc = bacc.Bacc("TRN2", target_bir_lowering=False, debug=False,
                   num_devices=num_devices)
    io = {}
    io["mask"] = nc.dram_tensor("mask", (L,), F32, kind="ExternalInput").ap()
    for name, shape in _W_NAMES:
        io[name] = nc.dram_tensor(name, shape, BF16, kind="ExternalInput").ap()
    io["out"] = nc.dram_tensor("out", (L, F), F32, kind="ExternalOutput").ap()
    with tile.TileContext(nc) as tc:
        _emit(tc, io)
    nc.compile()
    return nc


_NC_CACHE = []


def _build():
    if _NC_CACHE:
        return _NC_CACHE[0]
    nc = _build_module(NCORES)
    _NC_CACHE.append(nc)
    return nc


def _dev_mat(w):
    """(K, M) in-first weight -> device layout (128, K/128 * M)."""
    K, M = w.shape
    return w.reshape(K // 128, 128, M).transpose(1, 0, 2).reshape(128, -1)


def _prep_weights(inputs):
    """Host-side weight precombination (float64) + device-layout packing."""
    g = {k: np.asarray(v, np.float64) for k, v in inputs.items()}
    h = H
    Wfe1T = g["W_fe"][:, :h].T           # (h, h)
    U1 = g["W_ue"][:, :h].T
    U2 = g["W_ue"][:, h:].T
    M1 = Wfe1T @ U1
    M0 = M1 + Wfe1T @ U2
    A = g["W_attn"].T
    mats = {
        # xh is computed as true-xh/2 on device (prefolds the tanh half
        # angle): A_agg and b_agg are halved, A_uv1 doubled to compensate.
        "A_agg": 0.5 * g["W_agg"].T,
        "G1": M0 @ A,
        "G2": M1 @ A,
        "G3": M0 @ U2 @ A,
        "G4": M1 @ U2 @ A,
        "G5": M0 @ U2 @ U2 @ A,
        "A_uv1": 2.0 * g["W_uv"][:, :h].T,
        # sigmoid-as-tanh folding: sigma(s) = 0.5 + 0.5*tanh(s/2); the kernel
        # computes tp = tanh(-s/2), so A_uv2n = -0.5*A_uv2 and the constant
        # half goes into the bias.
        "A_uv2n": -0.5 * g["W_uv"][:, h:].T,
    }
    b_uvp = g["b_uv"] + 0.5 * g["W_uv"][:, h:].sum(axis=1)

    def seg(order):
        return np.concatenate([_dev_mat(mats[nm]) for nm in order], axis=1)

    segA_w = np.concatenate(
        [_dev_mat(g["W_inp"].T)]
        + [v.reshape(HH, 128).T for v in (g["b_inp"], 0.5 * g["b_agg"], b_uvp)],
        axis=1,
    )
    w = {
        "segA_w": segA_w,
        "segB": seg(_SEGB_ORDER), "segC": seg(_SEGC_ORDER),
        "segD": seg(_SEGD_ORDER), "segE": seg(_SEGE_ORDER),
        "segF": _dev_mat(g["W_oup"].T),
        "b_oup_row": g["b_oup"][None, :],
    }
    return {k: np.ascontiguousarray(v.astype(NPBF16)) for k, v in w.items()}


def _make_in_maps(inputs):
    """Full inputs -> per-core input dicts (host packing, incl. featT)."""
    w = _prep_weights(inputs)
    segA_w = w.pop("segA_w")
    feat = np.asarray(inputs["feat"], np.float32)
    mask = np.ascontiguousarray(np.asarray(inputs["mask"], np.float32))
    assert feat.shape == (NCORES, L, F), feat.shape
    in_maps = []
    for c in range(NCORES):
        featT = _dev_mat(feat[c].T).astype(NPBF16)
        segA = np.ascontiguousarray(np.concatenate([segA_w, featT], axis=1))
        im = {"segA": segA, "mask": mask[c]}
        im.update(w)
        in_maps.append(im)
    return in_maps


def kernel(**inputs) -> np.ndarray:
    nc = _build()
    in_maps = _make_in_maps(inputs)
    res = bass_utils.run_bass_kernel_spmd(nc, in_maps, core_ids=list(range(NCORES)))
    out = np.stack([res.results[c]["out"] for c in range(NCORES)], axis=0)
    return out.astype(np.float32)


if __name__ == "__main__":
    rng = np.random.default_rng(0)
    demo = {
        "feat": rng.standard_normal((NCORES, L, F)).astype(np.float32),
        "mask": np.ones((NCORES, L), np.float32),
    }
    for nm, shape in [("W_inp", (H, F)), ("b_inp", (H,)), ("W_oup", (F, H)),
                      ("b_oup", (F,)), ("W_fe", (H, 2 * H)), ("b_fe", (H,)),
                      ("W_ue", (H, 2 * H)), ("b_ue", (H,)), ("W_agg", (H, H)),
                      ("b_agg", (H,)), ("W_uv", (H, 2 * H)), ("b_uv", (H,)),
                      ("W_attn", (H, H)), ("b_attn", (H,))]:
        demo[nm] = (rng.standard_normal(shape) * 0.05).astype(np.float32)
    y = kernel(**demo)
    print("kernel output:", y.shape, y.dtype)
